# revision 7
# baseline (speedup 1.0000x reference)
"""AlphaGNN (2-layer GAT) on 8 TRN2 NeuronCores.

Strategy:
- Nodes partitioned across 8 cores (12500 each), sorted by in-degree desc,
  padded to 12544 = 98*128 rows per core.
- Edges live in a degree-grid: node rows on SBUF partitions, neighbor slots
  on the free dim. Slots hold the packed bf16 (k|v) row of the edge's src,
  fetched with one indirect DMA gather per supertile from an all-gathered
  [100352, 128] bf16 kv table (int32 offsets).
- Scores: DVE mul (k * q broadcast), then TensorE identity-matmul reduce
  over feature dim into cyclic-4 PSUM partials, folded by a tiny DVE reduce.
- Softmax: e = max(exp(s/8), 1) (== exp(relu(s)/8)); pad slots point at a
  zero row so e_pad = 1, corrected by per-node pad counts.
- Aggregation: DVE mul (v * e broadcast), TensorE identity-matmul reduce
  over neighbor slots; normalize by 1/denom.
- Inter-layer: AllGather of the per-core packed kv shard.
"""
import sys
import numpy as np

if "/opt/trn_rl_repo" not in sys.path:
    sys.path.insert(0, "/opt/trn_rl_repo")

import ml_dtypes  # noqa: E402
from concourse import bass, bacc, tile, mybir  # noqa: E402
from concourse.bass_utils import run_bass_kernel_spmd  # noqa: E402
from concourse.masks import make_identity  # noqa: E402
import concourse.bass_utils as _bu  # noqa: E402

_bu.upload_artifacts = lambda tmpdir: f"local://{tmpdir}"

dt = mybir.dt
BF = ml_dtypes.bfloat16

N, E, F, H = 100000, 1600000, 256, 64
C = 8                  # cores
NPC = 12500            # nodes per core
P = 128
NT = 98                # node tiles per core (98*128 = 12544)
NPAD = NT * P          # 12544
SD_CAP = 96            # max slots per partition per supertile
SMAX = 4               # max node tiles per supertile
G = 4                  # cyclic psum interleave for the feature reduce


def _bf16(x):
    return np.asarray(x, dtype=BF)


def preprocess(edge_index):
    """Host-side graph prep. Returns per-core device arrays + unpack info."""
    src = np.asarray(edge_index[0], np.int64)
    dst = np.asarray(edge_index[1], np.int64)
    deg = np.bincount(dst, minlength=N)

    # node permutation: per core, sort nodes by degree desc (stable)
    perm = np.empty(C * NPAD, np.int64)       # padded row -> node id (or -1)
    perm.fill(-1)
    inv_row = np.empty(N, np.int64)           # node id -> padded table row
    for c in range(C):
        nodes = np.arange(c * NPC, (c + 1) * NPC)
        order = nodes[np.argsort(-deg[nodes], kind="stable")]
        perm[c * NPAD: c * NPAD + NPC] = order
        inv_row[order] = c * NPAD + np.arange(NPC)
    SENT = NPAD - 1                           # any pad row of core 0 (zeros)

    # tile max degrees (max over cores at same tile position), mult of 4, >=4
    degp = np.zeros(C * NPAD, np.int64)
    valid = perm >= 0
    degp[valid] = deg[perm[valid]]
    degt = degp.reshape(C, NT, P)
    d_t = degt.max(axis=(0, 2))               # [NT]
    d_t = np.maximum(((d_t + 3) // 4) * 4, 4).astype(np.int64)

    # supertile grouping: pack tiles while count<=SMAX and count*maxD<=SD_CAP
    groups = []                               # list of (tile_start, n_tiles, D)
    t = 0
    while t < NT:
        dmax = d_t[t]
        cnt = 1
        while (t + cnt < NT and cnt < SMAX
               and (cnt + 1) * max(dmax, d_t[t + cnt]) <= SD_CAP):
            dmax = max(dmax, d_t[t + cnt])
            cnt += 1
        groups.append((t, cnt, int(dmax)))
        t += cnt
    TOT = sum(P * s * d for (_, s, d) in groups)

    # per-core offsets + padminus + edge->slot mapping
    # edges sorted by dst gives contiguous per-node runs
    e_order = np.argsort(dst, kind="stable")
    run_start = np.zeros(N + 1, np.int64)
    np.cumsum(np.bincount(dst, minlength=N), out=run_start[1:])

    offs = np.zeros((C, TOT), np.int32)
    padminus = np.zeros((C, NT * P), np.float32)
    edge_pos = np.empty(E, np.int64)          # edge -> flat slot pos (own core)
    edge_core = dst // NPC

    tbl_row = inv_row[src]                    # per edge, src's table row
    for c in range(C):
        base = 0
        for (t0, s_cnt, D) in groups:
            sd = s_cnt * D
            blk = np.full((P, sd), SENT, np.int32)
            for si in range(s_cnt):
                rows = (t0 + si) * P + np.arange(P)          # padded rows
                nodes_r = perm[c * NPAD + rows]
                for pi in range(P):
                    nd = nodes_r[pi]
                    if nd < 0:
                        continue
                    a, b = run_start[nd], run_start[nd + 1]
                    eids = e_order[a:b]
                    k = b - a
                    blk[pi, si * D: si * D + k] = tbl_row[eids]
                    edge_pos[eids] = base + pi * sd + si * D + np.arange(k)
            offs[c, base: base + P * sd] = blk.reshape(-1)
            base += P * sd
        dgc = degp[c * NPAD: (c + 1) * NPAD].reshape(NT, P)
        for gi, (t0, s_cnt, D) in enumerate(groups):
            for si in range(s_cnt):
                padminus[c, (t0 + si) * P: (t0 + si + 1) * P] = \
                    (D - dgc[t0 + si]) - 1e-6
    return dict(perm=perm, inv_row=inv_row, groups=groups, TOT=TOT,
                offs=offs, padminus=padminus, edge_pos=edge_pos,
                edge_core=edge_core, SENT=SENT)


def build(groups, TOT):
    nc = bacc.Bacc("TRN2", target_bir_lowering=False, debug=False,
                   num_devices=C)
    TBL = C * NPAD                                       # 100352

    xT = nc.dram_tensor("xT", [F, NPAD], dt.bfloat16, kind="ExternalInput")
    wproj = nc.dram_tensor("wproj", [F, H], dt.bfloat16, kind="ExternalInput")
    bproj = nc.dram_tensor("bproj", [H, 1], dt.float32, kind="ExternalInput")
    wkv1 = nc.dram_tensor("wkv1", [H, 2 * H], dt.bfloat16, kind="ExternalInput")
    wq1 = nc.dram_tensor("wq1", [H, H], dt.bfloat16, kind="ExternalInput")
    wkv2 = nc.dram_tensor("wkv2", [H, 2 * H], dt.bfloat16, kind="ExternalInput")
    wq2 = nc.dram_tensor("wq2", [H, H], dt.bfloat16, kind="ExternalInput")
    wsig = nc.dram_tensor("wsig", [H, 1], dt.bfloat16, kind="ExternalInput")
    bsig = nc.dram_tensor("bsig", [1, 1], dt.float32, kind="ExternalInput")
    offs_d = nc.dram_tensor("offs", [TOT], dt.int32, kind="ExternalInput")
    pmin_d = nc.dram_tensor("pmin", [NT * P], dt.float32, kind="ExternalInput")

    attn1_d = nc.dram_tensor("attn1", [TOT], dt.float32, kind="ExternalOutput")
    attn2_d = nc.dram_tensor("attn2", [TOT], dt.float32, kind="ExternalOutput")
    sig_d = nc.dram_tensor("sig", [1, NPAD], dt.float32, kind="ExternalOutput")

    kv1_in = nc.dram_tensor("kv1_in", [NPAD, 2 * H], dt.bfloat16,
                            kind="Internal")
    kv1_tab = nc.dram_tensor("kv1_tab", [TBL, 2 * H], dt.bfloat16,
                             kind="Internal", addr_space="Shared")
    kv2_in = nc.dram_tensor("kv2_in", [NPAD, 2 * H], dt.bfloat16,
                            kind="Internal")
    kv2_tab = nc.dram_tensor("kv2_tab", [TBL, 2 * H], dt.bfloat16,
                             kind="Internal", addr_space="Shared")

    RG = [list(range(C))]

    from contextlib import ExitStack
    with tile.TileContext(nc) as tc, ExitStack() as ctx:
        res = ctx.enter_context(tc.tile_pool(name="res", bufs=1))
        pool = ctx.enter_context(tc.tile_pool(name="work", bufs=2))
        psum = ctx.enter_context(tc.tile_pool(name="ps", bufs=2, space="PSUM"))
        psum1 = psum

        idb = res.tile([P, P], dt.bfloat16)
        make_identity(nc, idb[:])
        idf = res.tile([P, P], dt.float32)
        make_identity(nc, idf[:])

        # ---- params to SBUF (W_proj as two 128-row chunks)
        wp1 = res.tile([P, H], dt.bfloat16)
        wp2 = res.tile([P, H], dt.bfloat16)
        nc.sync.dma_start(out=wp1[:], in_=wproj[0:P, :])
        nc.sync.dma_start(out=wp2[:], in_=wproj[P:F, :])
        bp = res.tile([H, 1], dt.float32)
        nc.sync.dma_start(out=bp[:], in_=bproj[:])
        wkv1_t = res.tile([H, 2 * H], dt.bfloat16)
        nc.sync.dma_start(out=wkv1_t[:], in_=wkv1[:])
        wq1_t = res.tile([H, H], dt.bfloat16)
        nc.sync.dma_start(out=wq1_t[:], in_=wq1[:])
        wkv2_t = res.tile([H, 2 * H], dt.bfloat16)
        nc.sync.dma_start(out=wkv2_t[:], in_=wkv2[:])
        wq2_t = res.tile([H, H], dt.bfloat16)
        nc.sync.dma_start(out=wq2_t[:], in_=wq2[:])
        wsig_t = res.tile([H, 1], dt.bfloat16)
        nc.sync.dma_start(out=wsig_t[:], in_=wsig[:])
        bs = res.tile([1, 1], dt.float32)
        nc.sync.dma_start(out=bs[:], in_=bsig[:])
        pmin_t = res.tile([P, NT], dt.float32)
        nc.sync.dma_start(
            out=pmin_t[:],
            in_=pmin_d[:].rearrange("(t p) -> p t", t=NT, p=P))

        hT = res.tile([H, NPAD], dt.bfloat16)        # h0^T resident
        h1T = res.tile([H, NPAD], dt.bfloat16)       # gelu(h1)^T resident
        h2T = res.tile([H, NPAD], dt.bfloat16)       # h2^T resident
        q_rows = res.tile([P, NT * H], dt.bfloat16)  # q rows per tile block

        def proj_phase():
            CH = 448
            with nc.named_scope("proj"):
                for i in range(NPAD // CH):
                    xa = pool.tile([P, CH], dt.bfloat16, tag="xa")
                    xb = pool.tile([P, CH], dt.bfloat16, tag="xb")
                    nc.sync.dma_start(out=xa[:], in_=xT[0:P, i * CH:(i + 1) * CH])
                    nc.sync.dma_start(out=xb[:], in_=xT[P:F, i * CH:(i + 1) * CH])
                    ps = psum.tile([H, CH], dt.float32, space="PSUM", tag="tps")
                    nc.tensor.matmul(out=ps[:], lhsT=wp1[:], rhs=xa[:],
                                     start=True, stop=False)
                    nc.tensor.matmul(out=ps[:], lhsT=wp2[:], rhs=xb[:],
                                     start=False, stop=True)
                    nc.scalar.activation(
                        out=hT[:, i * CH:(i + 1) * CH], in_=ps[:],
                        func=mybir.ActivationFunctionType.Gelu, bias=bp[:])

        def build_tables(src_hT, wkv_t, wq_t, kv_in, kv_tab, lname):
            # q rows (transposed per tile) + kv rows -> kv_in -> AllGather
            with nc.named_scope(f"tab{lname}"):
                for t in range(NT):
                    sl = slice(t * P, (t + 1) * P)
                    qps = psum.tile([H, P], dt.float32, space="PSUM", tag="tps")
                    nc.tensor.matmul(out=qps[:], lhsT=wq_t[:],
                                     rhs=src_hT[:, sl], start=True, stop=True)
                    qsb = pool.tile([H, P], dt.bfloat16, tag="qsb")
                    nc.vector.tensor_copy(out=qsb[:], in_=qps[:])
                    qtp = psum1.tile([P, H], dt.bfloat16, space="PSUM",
                                     tag="tp")
                    nc.tensor.transpose(out=qtp[:], in_=qsb[:], identity=idb[0:H, 0:H])
                    nc.scalar.copy(out=q_rows[:, t * H:(t + 1) * H],
                                   in_=qtp[:])

                    kps = psum.tile([P, P], dt.float32, space="PSUM", tag="tps")
                    nc.tensor.matmul(out=kps[:], lhsT=wkv_t[:],
                                     rhs=src_hT[:, sl], start=True, stop=True)
                    ksb = pool.tile([P, P], dt.bfloat16, tag="ksb")
                    nc.vector.tensor_copy(out=ksb[:], in_=kps[:])
                    ktp = psum1.tile([P, P], dt.bfloat16, space="PSUM",
                                     tag="tp")
                    nc.tensor.transpose(out=ktp[:], in_=ksb[:], identity=idb[:])
                    krow = pool.tile([P, P], dt.bfloat16, tag="krow")
                    nc.scalar.copy(out=krow[:], in_=ktp[:])
                    nc.sync.dma_start(out=kv_in[sl, :], in_=krow[:])
            with nc.named_scope(f"ag{lname}"):
                nc.gpsimd.collective_compute(
                    "AllGather", mybir.AluOpType.bypass, replica_groups=RG,
                    ins=[kv_in[:].opt()], outs=[kv_tab[:].opt()])
                tc.strict_bb_all_engine_barrier()

        def edge_phase(kv_tab, attn_d, lname, gelu_out):
            # returns per-tile h rows written into hrows_res (via callback)
            base = 0
            with nc.named_scope(f"edge{lname}"):
                for gi, (t0, S, D) in enumerate(groups):
                    sd = S * D
                    off_t = pool.tile([P, sd], dt.int32, tag="off")
                    nc.sync.dma_start(
                        out=off_t[:],
                        in_=offs_d[base:base + P * sd]
                        .rearrange("(p w) -> p w", p=P, w=sd))
                    g = pool.tile([P, sd * 2 * H], dt.bfloat16, tag="g")
                    nc.gpsimd.indirect_dma_start(
                        out=g[:], out_offset=None, in_=kv_tab[:],
                        in_offset=bass.IndirectOffsetOnAxis(ap=off_t[:], axis=0))
                    gv = g[:].rearrange("p (s d c) -> p s d c", s=S, d=D,
                                        c=2 * H)
                    # scores mul: m = k * q  (q broadcast over slots)
                    qv = q_rows[:, t0 * H:(t0 + S) * H] \
                        .rearrange("p (s f) -> p s f", s=S, f=H) \
                        .unsqueeze(2).to_broadcast([P, S, D, H])
                    m = pool.tile([P, sd * H], dt.bfloat16, tag="m")
                    mv = m[:].rearrange("p (s d f) -> p s d f", s=S, d=D, f=H)
                    nc.vector.tensor_tensor(out=mv, in0=gv[:, :, :, 0:H],
                                            in1=qv, op=mybir.AluOpType.mult)
                    # scores reduce: cyclic-G psum over feature dim
                    sps = psum.tile([P, sd * G], dt.float32, space="PSUM",
                                    tag="sps")
                    rhs_s = m[:].rearrange(
                        "p (u chi clo) -> p u chi clo", u=sd, chi=H // G, clo=G)
                    out_s = sps[:].rearrange("p (u g) -> p u g", u=sd, g=G) \
                        .unsqueeze(2).to_broadcast([P, sd, H // G, G])
                    UC = 512 // H
                    for u0 in range(0, sd, UC):
                        u1 = min(u0 + UC, sd)
                        nc.tensor.matmul(out=out_s[:, u0:u1], lhsT=idb[:],
                                         rhs=rhs_s[:, u0:u1],
                                         start=True, stop=True,
                                         skip_group_check=True)
                    # fold G + exp + max(1)
                    s_sb = pool.tile([P, sd], dt.float32, tag="ssb")
                    nc.vector.tensor_reduce(
                        out=s_sb[:],
                        in_=sps[:].rearrange("p (u g) -> p u g", u=sd, g=G),
                        axis=mybir.AxisListType.X, op=mybir.AluOpType.add)
                    e_f = pool.tile([P, sd], dt.float32, tag="ef")
                    nc.scalar.activation(out=e_f[:], in_=s_sb[:],
                                         func=mybir.ActivationFunctionType.Exp,
                                         scale=0.125)
                    nc.vector.tensor_scalar_max(out=e_f[:], in0=e_f[:],
                                                scalar1=1.0)
                    # denom, reciprocal
                    dsum = pool.tile([P, S], dt.float32, tag="dsum")
                    nc.vector.tensor_reduce(
                        out=dsum[:],
                        in_=e_f[:].rearrange("p (s d) -> p s d", s=S, d=D),
                        axis=mybir.AxisListType.X, op=mybir.AluOpType.add)
                    nc.vector.tensor_tensor(out=dsum[:], in0=dsum[:],
                                            in1=pmin_t[:, t0:t0 + S],
                                            op=mybir.AluOpType.subtract)
                    rd = pool.tile([P, S], dt.float32, tag="rd")
                    nc.vector.reciprocal(out=rd[:], in_=dsum[:])
                    # attention out: a = e * rd
                    a_t = pool.tile([P, sd], dt.float32, tag="at")
                    nc.vector.tensor_tensor(
                        out=a_t[:].rearrange("p (s d) -> p s d", s=S, d=D),
                        in0=e_f[:].rearrange("p (s d) -> p s d", s=S, d=D),
                        in1=rd[:].unsqueeze(2).to_broadcast([P, S, D]),
                        op=mybir.AluOpType.mult)
                    nc.sync.dma_start(
                        out=attn_d[base:base + P * sd]
                        .rearrange("(p w) -> p w", p=P, w=sd), in_=a_t[:])
                    # aggregation: W = v * e ; reduce over slots
                    e_b = pool.tile([P, sd], dt.bfloat16, tag="eb")
                    nc.vector.tensor_copy(out=e_b[:], in_=e_f[:])
                    wgt = pool.tile([P, sd * H], dt.bfloat16, tag="wg")
                    wv = wgt[:].rearrange("p (s d f) -> p s d f", s=S, d=D, f=H)
                    nc.vector.tensor_tensor(
                        out=wv, in0=gv[:, :, :, H:2 * H],
                        in1=e_b[:].rearrange("p (s d) -> p s d", s=S, d=D)
                        .unsqueeze(3).to_broadcast([P, S, D, H]),
                        op=mybir.AluOpType.mult)
                    aps = psum1.tile([P, S * H], dt.float32, space="PSUM",
                                     tag="aps")
                    out_a = aps[:].rearrange("p (s f) -> p s f", s=S, f=H) \
                        .unsqueeze(2).to_broadcast([P, S, D, H])
                    DC = 512 // H
                    for si in range(S):
                        for d0 in range(0, D, DC):
                            d1 = min(d0 + DC, D)
                            nc.tensor.matmul(
                                out=out_a[:, si, d0:d1], lhsT=idb[:],
                                rhs=wv[:, si, d0:d1],
                                start=(d0 == 0), stop=(d1 >= D),
                                skip_group_check=True)
                    # normalize h = out_u * rd ; optional gelu; transpose
                    h_r = pool.tile([P, S * H], dt.float32, tag="hr")
                    nc.vector.tensor_tensor(
                        out=h_r[:].rearrange("p (s f) -> p s f", s=S, f=H),
                        in0=aps[:].rearrange("p (s f) -> p s f", s=S, f=H),
                        in1=rd[:].unsqueeze(2).to_broadcast([P, S, H]),
                        op=mybir.AluOpType.mult)
                    hb = pool.tile([P, S * H], dt.bfloat16, tag="hb")
                    if gelu_out:
                        nc.scalar.activation(
                            out=hb[:], in_=h_r[:],
                            func=mybir.ActivationFunctionType.Gelu)
                    else:
                        nc.scalar.copy(out=hb[:], in_=h_r[:])
                    dstT = h1T if gelu_out else h2T
                    for si in range(S):
                        htp = psum1.tile([H, P], dt.bfloat16, space="PSUM",
                                         tag="tp")
                        nc.tensor.transpose(
                            out=htp[:], in_=hb[:, si * H:(si + 1) * H],
                            identity=idb[:])
                        nc.scalar.copy(
                            out=dstT[:, (t0 + si) * P:(t0 + si + 1) * P],
                            in_=htp[:])
                    base += P * sd

        proj_phase()
        build_tables(hT, wkv1_t, wq1_t, kv1_in, kv1_tab, "1")
        edge_phase(kv1_tab, attn1_d, "1", gelu_out=True)
        build_tables(h1T, wkv2_t, wq2_t, kv2_in, kv2_tab, "2")
        edge_phase(kv2_tab, attn2_d, "2", gelu_out=False)

        with nc.named_scope("signals"):
            CH = 512
            for i in range(0, NPAD, CH):
                ch = min(CH, NPAD - i)
                ps = psum.tile([1, CH], dt.float32, space="PSUM", tag="tps")
                nc.tensor.matmul(out=ps[:, :ch], lhsT=wsig_t[:],
                                 rhs=h2T[:, i:i + ch], start=True, stop=True)
                ssb = pool.tile([1, CH], dt.float32, tag="sgsb")
                nc.scalar.activation(out=ssb[:, :ch], in_=ps[:, :ch],
                                     func=mybir.ActivationFunctionType.Identity,
                                     bias=bs[:])
                nc.sync.dma_start(out=sig_d[:, i:i + ch], in_=ssb[:, :ch])

    nc.compile()
    return nc


_GROUPS = None


def _run(inputs, trace=False):
    global _GROUPS
    x = np.asarray(inputs["x"], np.float32)
    edge_index = np.asarray(inputs["edge_index"])
    meta = preprocess(edge_index)
    groups, TOT = meta["groups"], meta["TOT"]
    _GROUPS = groups

    wkv1 = np.concatenate([inputs["Wk1"], inputs["Wv1"]], axis=1)
    wkv2 = np.concatenate([inputs["Wk2"], inputs["Wv2"]], axis=1)

    in_maps = []
    perm = meta["perm"]
    for c in range(C):
        rows = perm[c * NPAD:(c + 1) * NPAD]
        xp = np.zeros((NPAD, F), np.float32)
        v = rows >= 0
        xp[v] = x[rows[v]]
        in_maps.append({
            "xT": _bf16(xp.T.copy()),
            "wproj": _bf16(inputs["W_proj"]),
            "bproj": np.asarray(inputs["b_proj"], np.float32).reshape(H, 1),
            "wkv1": _bf16(wkv1), "wq1": _bf16(inputs["Wq1"]),
            "wkv2": _bf16(wkv2), "wq2": _bf16(inputs["Wq2"]),
            "wsig": _bf16(inputs["W_sig"]),
            "bsig": np.asarray(inputs["b_sig"], np.float32).reshape(1, 1),
            "offs": meta["offs"][c],
            "pmin": meta["padminus"][c],
        })

    nc = build(groups, TOT)
    res = run_bass_kernel_spmd(nc, in_maps, core_ids=list(range(C)),
                               trace=trace)

    # assemble outputs
    signals = np.zeros((N, 1), np.float32)
    for c in range(C):
        rows = perm[c * NPAD:(c + 1) * NPAD]
        v = rows >= 0
        signals[rows[v], 0] = res.results[c]["sig"][0][v]
    attn1 = np.zeros((E, 1), np.float32)
    attn2 = np.zeros((E, 1), np.float32)
    ec, ep = meta["edge_core"], meta["edge_pos"]
    for c in range(C):
        m = ec == c
        attn1[m, 0] = res.results[c]["attn1"][ep[m]]
        attn2[m, 0] = res.results[c]["attn2"][ep[m]]
    return (signals, attn1, attn2), res


def kernel(**inputs):
    outs, _ = _run(inputs, trace=False)
    return outs
